# revision 1
# baseline (speedup 1.0000x reference)
"""Trainium2 Bass kernel for nn_MOAB_46273977647401.

Network (reference):
  x1 (256,256), x3 (256,) -> 4 outer sigmoid maps (256,257,257)
  -> 1x1 conv combine (4ch) + eval BN + leaky(0.1) -> (256, 66049)
  -> FC (66049 -> 512) + relu -> FC (512 -> 4)

Sharding: 8 cores = 4 batch shards (Bc=64) x 2 H shards (Hc=256).
Per core, z maps are computed in [i-partitions, (j,b)-free] layout:
  z[i, (j,b)] = sigmoid(f(b0/b1[i], x1T[j,b]))
using PE row-broadcast of x1T into PSUM, ScalarE sigmoid with per-partition
bias/scale, DVE+Pool combine (conv+BN folded to 4 scales + offset, leaky),
feeding a K=66049 PSUM-accumulated matmul with host-relaid fc_w (i,j,h).
"""

import numpy as np

import concourse.bass as bass
import concourse.tile as tile
from concourse import bacc, mybir
from concourse.bass_utils import run_bass_kernel_spmd

F32 = mybir.dt.float32
BF16 = mybir.dt.bfloat16
AL = mybir.AluOpType

B, N, H, C = 256, 256, 512, 4
NP = 257                  # N+1
P_B, P_H = 4, 2           # batch shards x h shards
BC = B // P_B             # 64 batch rows per core
HC = H // P_H             # 256 fc outputs per core
JC = 16                   # j values per chunk (main loop, j in [1,257))
CH = JC * BC              # 1024 free elems per chunk
NCHUNK = 256 // JC        # 16
EPS = 1e-10
BN_EPS = 1e-5
LEAKY = 0.1

# W dtype for the big fc_w stream + matmul lhs (y) dtype.
W_DTYPE = BF16
W_NP = np.dtype(np.float32) if W_DTYPE == F32 else np.dtype("bfloat16")


def build_program():
    nc = bacc.Bacc("TRN2", target_bir_lowering=False, debug=False, num_devices=8)

    d_a0T = nc.dram_tensor("a0T", [NP, BC], F32, kind="ExternalInput").ap()
    d_a1T = nc.dram_tensor("a1T", [NP, BC], F32, kind="ExternalInput").ap()
    d_aflat = nc.dram_tensor("aflat", [NCHUNK, CH], F32, kind="ExternalInput").ap()
    d_b0 = nc.dram_tensor("b0", [NP, 1], F32, kind="ExternalInput").ap()
    d_b1 = nc.dram_tensor("b1", [NP, 1], F32, kind="ExternalInput").ap()
    d_cv = nc.dram_tensor("cv", [128, 1], F32, kind="ExternalInput").ap()
    d_sv = nc.dram_tensor("sv", [128, 6], F32, kind="ExternalInput").ap()
    d_w3 = nc.dram_tensor("w3", [NP, NP, HC], W_DTYPE, kind="ExternalInput").ap()
    d_wstrip = nc.dram_tensor("wstrip", [NP, HC], W_DTYPE, kind="ExternalInput").ap()
    d_wcol0 = nc.dram_tensor("wcol0", [NP, HC], W_DTYPE, kind="ExternalInput").ap()
    d_fcb = nc.dram_tensor("fcb", [HC, 1], F32, kind="ExternalInput").ap()
    d_owt = nc.dram_tensor("owt", [HC, C], F32, kind="ExternalInput").ap()
    d_eye = nc.dram_tensor("eye", [64, 64], F32, kind="ExternalInput").ap()
    d_out = nc.dram_tensor("out", [BC, C], F32, kind="ExternalOutput").ap()

    with tile.TileContext(nc) as tc:
        with (
            tc.tile_pool(name="const", bufs=1) as cpool,
            tc.tile_pool(name="setup", bufs=1) as spool,
            tc.tile_pool(name="stage", bufs=3) as stpool,
            tc.tile_pool(name="w0", bufs=3) as wpool0,
            tc.tile_pool(name="w1", bufs=3) as wpool1,
            tc.tile_pool(name="z", bufs=3) as zpool,
            tc.tile_pool(name="comb", bufs=3) as combpool,
            tc.tile_pool(name="ypool", bufs=3) as ypool,
            tc.tile_pool(name="fin", bufs=1) as finpool,
            tc.tile_pool(name="psA", bufs=2, space="PSUM") as psA,
            tc.tile_pool(name="psR", bufs=1, space="PSUM") as psR,
            tc.tile_pool(name="psO", bufs=1, space="PSUM") as psO,
            tc.tile_pool(name="psT", bufs=1, space="PSUM") as psT,
        ):
            # ---------------- constants / setup ----------------
            a0 = [cpool.tile([128, BC], F32, tag=f"a0_{k}", name=f"a0_{k}") for k in range(3)]
            a1 = [cpool.tile([128, BC], F32, tag=f"a1_{k}", name=f"a1_{k}") for k in range(3)]
            nc.sync.dma_start(a0[0][:, :], d_a0T[0:128, :])
            nc.sync.dma_start(a0[1][:, :], d_a0T[128:256, :])
            nc.sync.dma_start(a0[2][0:1, :], d_a0T[256:257, :])
            nc.sync.dma_start(a1[0][:, :], d_a1T[0:128, :])
            nc.sync.dma_start(a1[1][:, :], d_a1T[128:256, :])
            nc.sync.dma_start(a1[2][0:1, :], d_a1T[256:257, :])

            b0t = [cpool.tile([128, 1], F32, tag=f"b0_{k}", name=f"b0_{k}") for k in range(2)]
            b1t = [cpool.tile([128, 1], F32, tag=f"b1_{k}", name=f"b1_{k}") for k in range(2)]
            nc.sync.dma_start(b0t[0][:, :], d_b0[0:128, :])
            nc.sync.dma_start(b0t[1][:, :], d_b0[128:256, :])
            nc.sync.dma_start(b1t[0][:, :], d_b1[0:128, :])
            nc.sync.dma_start(b1t[1][:, :], d_b1[128:256, :])

            cv = cpool.tile([128, 1], F32, tag="cv")
            sv = cpool.tile([128, 6], F32, tag="sv")
            nc.sync.dma_start(cv[:, :], d_cv[:, :])
            nc.sync.dma_start(sv[:, :], d_sv[:, :])

            fcb = [cpool.tile([128, 1], F32, tag=f"fcb_{k}", name=f"fcb_{k}") for k in range(2)]
            owt = [cpool.tile([128, C], F32, tag=f"owt_{k}", name=f"owt_{k}") for k in range(2)]
            nc.sync.dma_start(fcb[0][:, :], d_fcb[0:128, :])
            nc.sync.dma_start(fcb[1][:, :], d_fcb[128:256, :])
            nc.sync.dma_start(owt[0][:, :], d_owt[0:128, :])
            nc.sync.dma_start(owt[1][:, :], d_owt[128:256, :])

            eye = cpool.tile([64, 64], F32, tag="eye")
            nc.sync.dma_start(eye[:, :], d_eye[:, :])

            ones1 = cpool.tile([1, 128], F32, tag="ones1")
            nc.vector.memset(ones1[:, :], 1.0)

            # recip tiles for the i=256 strip: r = 1/(a1 + eps), (j,b) layout
            rt = [cpool.tile([128, BC], F32, tag=f"r_{k}", name=f"r_{k}") for k in range(3)]
            for k, npart in ((0, 128), (1, 128), (2, 1)):
                tmp = spool.tile([128, BC], F32, tag=f"rtmp_{k}")
                nc.vector.tensor_scalar_add(
                    tmp[0:npart, :], a1[k][0:npart, :], EPS
                )
                nc.vector.reciprocal(rt[k][0:npart, :], tmp[0:npart, :])

            # rflat16: recip of aflat rows, in [NCHUNK, CH] layout (on 16
            # partitions; only used as DMA source for per-chunk staging)
            af16 = spool.tile([NCHUNK, CH], F32, tag="af16")
            nc.sync.dma_start(af16[:, :], d_aflat[:, :])
            rf16 = cpool.tile([NCHUNK, CH], F32, tag="rf16")
            rtmp16 = spool.tile([NCHUNK, CH], F32, tag="rtmp16")
            nc.vector.tensor_scalar_add(rtmp16[:, :], af16[:, :], EPS)
            nc.vector.reciprocal(rf16[:, :], rtmp16[:, :])

            # ---------------- main accumulation ----------------
            psum_out = psO.tile([BC, HC], F32, tag="acc")
            mm_started = [False]

            def acc_mm(lhsT, rhs, stop=False):
                nc.tensor.matmul(
                    psum_out[:, :],
                    lhsT,
                    rhs,
                    start=not mm_started[0],
                    stop=stop,
                    skip_group_check=True,
                )
                mm_started[0] = True

            for c in range(NCHUNK):
                j0 = 1 + c * JC
                # stage a/r rows for this chunk on partition 0
                stg_a = stpool.tile([1, CH], F32, tag="stg_a")
                nc.sync.dma_start(stg_a[:, :], d_aflat[c : c + 1, :])
                stg_r = stpool.tile([1, CH], F32, tag="stg_r")
                nc.gpsimd.dma_start(stg_r[:, :], rf16[c : c + 1, :])

                # PE broadcast to 128 partitions (PSUM)
                arep = psA.tile([128, CH], F32, tag="arep")
                rrep = psR.tile([128, CH], F32, tag="rrep")
                for half in range(2):
                    sl = slice(half * 512, (half + 1) * 512)
                    nc.tensor.matmul(
                        arep[:, sl], ones1[:, :], stg_a[:, sl],
                        start=True, stop=True, skip_group_check=True,
                    )
                    nc.tensor.matmul(
                        rrep[:, sl], ones1[:, :], stg_r[:, sl],
                        start=True, stop=True, skip_group_check=True,
                    )

                # W slabs for both i-tiles
                wsl = []
                for it, wpool in ((0, wpool0), (1, wpool1)):
                    w = wpool.tile([128, JC * HC], W_DTYPE, tag=f"wsl{it}")
                    nc.sync.dma_start(
                        w[:, :], d_w3[it * 128 : (it + 1) * 128, j0 : j0 + JC, :]
                    )
                    wsl.append(w)

                for it in range(2):
                    SIG = mybir.ActivationFunctionType.Sigmoid
                    za = zpool.tile([128, CH], BF16, tag="za")
                    nc.scalar.activation(za[:, :], arep[:, :], SIG,
                                         bias=b0t[it][:, :], scale=1.0)
                    zs = zpool.tile([128, CH], BF16, tag="zs")
                    nc.scalar.activation(zs[:, :], arep[:, :], SIG,
                                         bias=b0t[it][:, :], scale=-1.0)
                    zp = zpool.tile([128, CH], BF16, tag="zp")
                    nc.scalar.activation(zp[:, :], arep[:, :], SIG,
                                         bias=0.0, scale=b1t[it][:, :])
                    zd = zpool.tile([128, CH], BF16, tag="zd")
                    nc.scalar.activation(zd[:, :], rrep[:, :], SIG,
                                         bias=0.0, scale=b1t[it][:, :])

                    # y = s0*za + s1*zs + s2*zp + s3*zd + off ; leaky
                    # all-bf16 DVE chain: ts gets 4x mode, stt gets 2x
                    # ts (4x bf16) scale passes + tt (2x bf16) adds beat
                    # stt chains (stt has no bf16 2x uop -> 1x)
                    ta = combpool.tile([128, CH], BF16, tag="ta")
                    nc.vector.tensor_scalar(ta[:, :], za[:, :],
                                            sv[:, 0:1], sv[:, 4:5],
                                            AL.mult, AL.add)
                    tb = combpool.tile([128, CH], BF16, tag="tb")
                    nc.vector.tensor_scalar(tb[:, :], zs[:, :],
                                            sv[:, 1:2], None, AL.mult)
                    tc2 = combpool.tile([128, CH], BF16, tag="tc2")
                    nc.vector.tensor_scalar(tc2[:, :], zp[:, :],
                                            sv[:, 2:3], None, AL.mult)
                    td = combpool.tile([128, CH], BF16, tag="td")
                    nc.vector.tensor_scalar(td[:, :], zd[:, :],
                                            sv[:, 3:4], None, AL.mult)
                    u1 = combpool.tile([128, CH], BF16, tag="u1")
                    nc.vector.tensor_add(u1[:, :], ta[:, :], tb[:, :])
                    u2 = combpool.tile([128, CH], BF16, tag="u2")
                    nc.vector.tensor_add(u2[:, :], tc2[:, :], td[:, :])
                    y1 = combpool.tile([128, CH], BF16, tag="y1")
                    nc.vector.tensor_add(y1[:, :], u1[:, :], u2[:, :])
                    lk = combpool.tile([128, CH], BF16, tag="lk")
                    nc.vector.tensor_scalar(lk[:, :], y1[:, :],
                                            LEAKY, None, AL.mult)
                    yl = ypool.tile([128, CH], W_DTYPE, tag="yl")
                    nc.vector.tensor_tensor(yl[:, :], y1[:, :], lk[:, :],
                                            AL.max)

                    for jw in range(JC):
                        acc_mm(
                            yl[:, jw * BC : (jw + 1) * BC],
                            wsl[it][:, jw * HC : (jw + 1) * HC],
                        )

            # ---------------- strip j=0 (i in [0,256)) ----------------
            SIG = mybir.ActivationFunctionType.Sigmoid
            for it in range(2):
                za0 = spool.tile([128, 1], F32, tag=f"za0_{it}")
                nc.scalar.activation(za0[:, :], b0t[it][:, :], SIG)
                zp0 = spool.tile([128, 1], F32, tag=f"zp0_{it}")
                nc.scalar.activation(zp0[:, :], b1t[it][:, :], SIG)
                zd0 = spool.tile([128, 1], F32, tag=f"zd0_{it}")
                nc.scalar.activation(zd0[:, :], b1t[it][:, :], SIG,
                                     bias=0.0, scale=1.0 / (1.0 + EPS))
                tt = spool.tile([128, 1], F32, tag=f"tt0_{it}")
                nc.vector.tensor_scalar(tt[:, :], za0[:, :],
                                        sv[:, 5:6], sv[:, 4:5],
                                        AL.mult, AL.add)
                nc.vector.scalar_tensor_tensor(tt[:, :], zp0[:, :],
                                               sv[:, 2:3], tt[:, :],
                                               AL.mult, AL.add)
                nc.vector.scalar_tensor_tensor(tt[:, :], zd0[:, :],
                                               sv[:, 3:4], tt[:, :],
                                               AL.mult, AL.add)
                yl0 = spool.tile([128, 1], F32, tag=f"yl0_{it}")
                nc.vector.scalar_tensor_tensor(yl0[:, :], tt[:, :],
                                               LEAKY, tt[:, :],
                                               AL.mult, AL.max)
                yj0 = spool.tile([128, BC], W_DTYPE, tag=f"yj0_{it}")
                nc.vector.tensor_copy(yj0[:, :],
                                      yl0[:, 0:1].broadcast_to([128, BC]))
                wj0 = spool.tile([128, HC], W_DTYPE, tag=f"wj0_{it}")
                nc.sync.dma_start(wj0[:, :],
                                  d_wcol0[it * 128 : (it + 1) * 128, :])
                acc_mm(yj0[:, :], wj0[:, :])

            # ---------------- strip i=256 (j in [0,257)) ----------------
            for jt, (jof, jsz) in enumerate(((0, 128), (128, 128), (256, 1))):
                za = spool.tile([128, BC], F32, tag=f"sza_{jt}")
                nc.scalar.activation(za[0:jsz, :], a0[jt][0:jsz, :], SIG,
                                     bias=cv[0:jsz, :], scale=1.0)
                zs = spool.tile([128, BC], F32, tag=f"szs_{jt}")
                nc.scalar.activation(zs[0:jsz, :], a0[jt][0:jsz, :], SIG,
                                     bias=cv[0:jsz, :], scale=-1.0)
                zp = spool.tile([128, BC], F32, tag=f"szp_{jt}")
                nc.scalar.activation(zp[0:jsz, :], a1[jt][0:jsz, :], SIG,
                                     bias=0.0, scale=cv[0:jsz, :])
                zd = spool.tile([128, BC], F32, tag=f"szd_{jt}")
                nc.scalar.activation(zd[0:jsz, :], rt[jt][0:jsz, :], SIG,
                                     bias=0.0, scale=cv[0:jsz, :])
                t1 = spool.tile([128, BC], F32, tag=f"st1_{jt}")
                nc.vector.tensor_scalar(t1[0:jsz, :], za[0:jsz, :],
                                        sv[0:jsz, 0:1], sv[0:jsz, 4:5],
                                        AL.mult, AL.add)
                nc.vector.scalar_tensor_tensor(t1[0:jsz, :], zs[0:jsz, :],
                                               sv[0:jsz, 1:2], t1[0:jsz, :],
                                               AL.mult, AL.add)
                nc.vector.scalar_tensor_tensor(t1[0:jsz, :], zp[0:jsz, :],
                                               sv[0:jsz, 2:3], t1[0:jsz, :],
                                               AL.mult, AL.add)
                nc.vector.scalar_tensor_tensor(t1[0:jsz, :], zd[0:jsz, :],
                                               sv[0:jsz, 3:4], t1[0:jsz, :],
                                               AL.mult, AL.add)
                yls = spool.tile([128, BC], W_DTYPE, tag=f"syl_{jt}")
                nc.vector.scalar_tensor_tensor(yls[0:jsz, :], t1[0:jsz, :],
                                               LEAKY, t1[0:jsz, :],
                                               AL.mult, AL.max)
                ws = spool.tile([128, HC], W_DTYPE, tag=f"sws_{jt}")
                nc.sync.dma_start(ws[0:jsz, :], d_wstrip[jof : jof + jsz, :])
                acc_mm(yls[0:jsz, :], ws[0:jsz, :], stop=(jt == 2))

            # ---------------- tail: relu + fc2 ----------------
            y2 = finpool.tile([BC, HC], F32, tag="y2")
            nc.scalar.copy(y2[:, :], psum_out[:, :])
            lg = psO.tile([BC, C], F32, tag="acc")
            for h2 in range(2):
                yT = psT.tile([128, BC], F32, tag="yT")
                nc.tensor.transpose(yT[:, :],
                                    y2[:, h2 * 128 : (h2 + 1) * 128],
                                    eye[:, :])
                ryT = finpool.tile([128, BC], F32, tag=f"ryT_{h2}")
                nc.scalar.activation(ryT[:, :], yT[:, :],
                                     mybir.ActivationFunctionType.Relu,
                                     bias=fcb[h2][:, :], scale=1.0)
                nc.tensor.matmul(lg[:, :], ryT[:, :], owt[h2][:, :],
                                 start=(h2 == 0), stop=(h2 == 1),
                                 skip_group_check=True)
            outt = finpool.tile([BC, C], F32, tag="outt")
            nc.scalar.copy(outt[:, :], lg[:, :])
            nc.sync.dma_start(d_out[:, :], outt[:, :])

    nc.finalize()
    return nc


_CACHED_NC = None


def _get_program():
    global _CACHED_NC
    if _CACHED_NC is None:
        _CACHED_NC = build_program()
    return _CACHED_NC


def make_in_maps(x1, x3, conv_w, conv_b, bn_gamma, bn_beta, bn_mean, bn_var,
                 fc_w, fc_b, out_w, out_b):
    x1 = np.asarray(x1, np.float32)
    x3 = np.asarray(x3, np.float32)
    fc_w = np.asarray(fc_w, np.float32)

    g = float(np.asarray(bn_gamma).reshape(-1)[0]) / float(
        np.sqrt(np.asarray(bn_var).reshape(-1)[0] + BN_EPS))
    s = np.asarray(conv_w, np.float32).reshape(-1) * g
    off = (float(np.asarray(conv_b).reshape(-1)[0])
           - float(np.asarray(bn_mean).reshape(-1)[0])) * g \
        + float(np.asarray(bn_beta).reshape(-1)[0])

    sv = np.zeros((128, 6), np.float32)
    sv[:, 0], sv[:, 1], sv[:, 2], sv[:, 3] = s[0], s[1], s[2], s[3]
    sv[:, 4] = off
    sv[:, 5] = s[0] + s[1]
    cv = np.full((128, 1), x3[-1], np.float32)

    b0 = np.concatenate([[0.0], x3]).astype(np.float32).reshape(NP, 1)
    b1 = np.concatenate([[1.0], x3]).astype(np.float32).reshape(NP, 1)

    # fc_w (H, 66049) with k = i*257+j  ->  W3 [i, j, h]
    w3 = np.ascontiguousarray(
        fc_w.reshape(H, NP, NP).transpose(1, 2, 0))
    eye = np.eye(64, dtype=np.float32)

    x1T = np.ascontiguousarray(x1.T)  # (256 j, 256 b)

    in_maps = []
    for core in range(8):
        bp, hq = core // P_H, core % P_H
        xs = np.ascontiguousarray(x1T[:, bp * BC : (bp + 1) * BC])
        a0T = np.concatenate([np.zeros((1, BC), np.float32), xs])
        a1T = np.concatenate([np.ones((1, BC), np.float32), xs])
        aflat = np.ascontiguousarray(xs.reshape(NCHUNK, CH))
        hsl = slice(hq * HC, (hq + 1) * HC)
        w3q = np.ascontiguousarray(w3[:, :, hsl]).astype(W_NP)
        in_maps.append({
            "a0T": a0T, "a1T": a1T, "aflat": aflat,
            "b0": b0, "b1": b1, "cv": cv, "sv": sv,
            "w3": w3q,
            "wstrip": np.ascontiguousarray(w3q[256, :, :]),
            "wcol0": np.ascontiguousarray(w3q[:, 0, :]),
            "fcb": np.asarray(fc_b, np.float32)[hsl].reshape(HC, 1),
            "owt": np.ascontiguousarray(
                np.asarray(out_w, np.float32)[:, hsl].T),
            "eye": eye,
        })
    return in_maps


def kernel(**inputs):
    in_maps = make_in_maps(**inputs)
    nc = _get_program()
    res = run_bass_kernel_spmd(nc, in_maps, list(range(8)))

    out = np.zeros((B, C), np.float32)
    outb = np.asarray(inputs["out_b"], np.float32).reshape(1, C)
    for bp in range(P_B):
        acc = np.zeros((BC, C), np.float32)
        for hq in range(P_H):
            acc += res.results[bp * P_H + hq]["out"]
        out[bp * BC : (bp + 1) * BC] = acc + outb
    return out



# revision 3
# speedup vs baseline: 1.6559x; 1.6559x over previous
"""Trainium2 Bass kernel for nn_MOAB_46273977647401.

Network (reference):
  x1 (256,256), x3 (256,) -> 4 outer sigmoid maps (256,257,257)
  -> 1x1 conv combine (4ch) + eval BN + leaky(0.1) -> (256, 66049)
  -> FC (66049 -> 512) + relu -> FC (512 -> 4)

Sharding: 8-way split of the FC contraction dim K = 257*257 by the j
(column) index: core c owns j in [1+32c, 33+32c); the j=0 column and the
i=256 strip's j=0 element are computed identically on every core against
1/8-scaled weights (outputs are partial sums, reduced on host).

Per core, z maps live in [i-partitions, (j,b)-free] layout. The a row
values (and their host-precomputed reciprocals) are DMA-broadcast from
DRAM to all 128 partitions as bf16; ScalarE computes the 4 sigmoid maps
with per-partition bias/scale; DVE does the conv+BN-folded combine with
the leaky-relu max on the Pool engine; PE accumulates out[h, b] PSUM
tiles with the fc_w slab as the stationary (lhsT) operand.

Host: sums the 8 partial (512, 256) outputs, applies fc bias + relu and
the tiny 512->4 output layer.
"""

import numpy as np

import concourse.bass as bass
import concourse.tile as tile
from concourse import bacc, mybir
from concourse.bass_utils import run_bass_kernel_spmd

F32 = mybir.dt.float32
BF16 = mybir.dt.bfloat16
AL = mybir.AluOpType
SIG = mybir.ActivationFunctionType.Sigmoid

B, N, H, C = 256, 256, 512, 4
NP = 257                  # N+1
NCORE = 8
JPC = 32                  # j columns per core (j in [1+32c, 33+32c))
JC = 8                    # j values per chunk
CH = JC * B               # 2048 free elems per chunk
NCHUNK = JPC // JC        # 4
NHT = H // 128            # 4 output h tiles
EPS = 1e-10
BN_EPS = 1e-5
LEAKY = 0.1


def build_program():
    nc = bacc.Bacc("TRN2", target_bir_lowering=False, debug=False, num_devices=8)

    d_arows = nc.dram_tensor("arows", [1, JPC * B], BF16, kind="ExternalInput").ap()
    d_rrows = nc.dram_tensor("rrows", [1, JPC * B], BF16, kind="ExternalInput").ap()
    d_b0 = nc.dram_tensor("b0", [2 * 128, 1], F32, kind="ExternalInput").ap()
    d_b1 = nc.dram_tensor("b1", [2 * 128, 1], F32, kind="ExternalInput").ap()
    d_sv = nc.dram_tensor("sv", [128, 8], F32, kind="ExternalInput").ap()
    d_cv = nc.dram_tensor("cv", [128, 1], F32, kind="ExternalInput").ap()
    d_wmain = nc.dram_tensor("wmain", [2 * NCHUNK * 128, JC * H], BF16,
                             kind="ExternalInput").ap()
    d_wcol0 = nc.dram_tensor("wcol0", [2 * 128, H], BF16, kind="ExternalInput").ap()
    d_wstrip = nc.dram_tensor("wstrip", [JPC + 1, H], BF16, kind="ExternalInput").ap()
    d_a0s = nc.dram_tensor("a0s", [JPC + 1, B], BF16, kind="ExternalInput").ap()
    d_a1s = nc.dram_tensor("a1s", [JPC + 1, B], BF16, kind="ExternalInput").ap()
    d_rs = nc.dram_tensor("rs", [JPC + 1, B], BF16, kind="ExternalInput").ap()
    d_out = nc.dram_tensor("out", [H, B], F32, kind="ExternalOutput").ap()

    JS = JPC + 1  # strip partitions

    with tile.TileContext(nc) as tc:
        with (
            tc.tile_pool(name="const", bufs=1) as cpool,
            tc.tile_pool(name="sp", bufs=1) as spool,
            tc.tile_pool(name="w", bufs=3) as wpool,
            tc.tile_pool(name="ar", bufs=2) as arpool,
            tc.tile_pool(name="z", bufs=2) as zpool,
            tc.tile_pool(name="comb", bufs=2) as combpool,
            tc.tile_pool(name="ylp", bufs=2) as ylpool,
            tc.tile_pool(name="fin", bufs=1) as finpool,
            tc.tile_pool(name="psO", bufs=1, space="PSUM") as psO,
        ):
            # ---------------- constants ----------------
            b0t = [cpool.tile([128, 1], F32, tag=f"b0_{k}", name=f"b0_{k}")
                   for k in range(2)]
            b1t = [cpool.tile([128, 1], F32, tag=f"b1_{k}", name=f"b1_{k}")
                   for k in range(2)]
            for k in range(2):
                nc.sync.dma_start(b0t[k][:, :], d_b0[k * 128:(k + 1) * 128, :])
                nc.sync.dma_start(b1t[k][:, :], d_b1[k * 128:(k + 1) * 128, :])
            sv = cpool.tile([128, 8], F32, tag="sv")
            nc.sync.dma_start(sv[:, :], d_sv[:, :])
            cv = cpool.tile([128, 1], F32, tag="cv")
            nc.sync.dma_start(cv[:, :], d_cv[:, :])

            wc0 = [cpool.tile([128, H], BF16, tag=f"wc0_{k}", name=f"wc0_{k}")
                   for k in range(2)]
            for k in range(2):
                nc.sync.dma_start(wc0[k][:, :], d_wcol0[k * 128:(k + 1) * 128, :])
            wst = cpool.tile([JS, H], BF16, tag="wst")
            nc.sync.dma_start(wst[:, :], d_wstrip[:, :])
            a0s = cpool.tile([JS, B], BF16, tag="a0s")
            a1s = cpool.tile([JS, B], BF16, tag="a1s")
            rs = cpool.tile([JS, B], BF16, tag="rs")
            nc.sync.dma_start(a0s[:, :], d_a0s[:, :])
            nc.sync.dma_start(a1s[:, :], d_a1s[:, :])
            nc.sync.dma_start(rs[:, :], d_rs[:, :])

            # persistent PSUM accumulators out[h, b]
            ot = [psO.tile([128, B], F32, tag=f"ot{h}", name=f"ot{h}")
                  for h in range(NHT)]

            def mm(ht, lhsT, rhs, start, stop):
                nc.tensor.matmul(ot[ht][:, :], lhsT, rhs,
                                 start=start, stop=stop, skip_group_check=True)

            # ---------------- strip i=256 (j in core shard + j0/8) --------
            zas = spool.tile([JS, B], F32, tag="zas")
            nc.scalar.activation(zas[:, :], a0s[:, :], SIG,
                                 bias=cv[0:JS, :], scale=1.0)
            zss = spool.tile([JS, B], F32, tag="zss")
            nc.scalar.activation(zss[:, :], a0s[:, :], SIG,
                                 bias=cv[0:JS, :], scale=-1.0)
            zps = spool.tile([JS, B], F32, tag="zps")
            nc.scalar.activation(zps[:, :], a1s[:, :], SIG,
                                 bias=0.0, scale=cv[0:JS, :])
            zds = spool.tile([JS, B], F32, tag="zds")
            nc.scalar.activation(zds[:, :], rs[:, :], SIG,
                                 bias=0.0, scale=cv[0:JS, :])
            t1 = spool.tile([JS, B], F32, tag="t1")
            nc.vector.tensor_scalar(t1[:, :], zas[:, :],
                                    sv[0:JS, 0:1], sv[0:JS, 4:5],
                                    AL.mult, AL.add)
            nc.vector.scalar_tensor_tensor(t1[:, :], zss[:, :],
                                           sv[0:JS, 1:2], t1[:, :],
                                           AL.mult, AL.add)
            nc.vector.scalar_tensor_tensor(t1[:, :], zps[:, :],
                                           sv[0:JS, 2:3], t1[:, :],
                                           AL.mult, AL.add)
            nc.vector.scalar_tensor_tensor(t1[:, :], zds[:, :],
                                           sv[0:JS, 3:4], t1[:, :],
                                           AL.mult, AL.add)
            yls = spool.tile([JS, B], BF16, tag="yls")
            nc.vector.scalar_tensor_tensor(yls[:, :], t1[:, :],
                                           LEAKY, t1[:, :],
                                           AL.mult, AL.max)
            for ht in range(NHT):
                mm(ht, wst[:, ht * 128:(ht + 1) * 128], yls[:, :],
                   start=True, stop=False)

            # ---------------- j=0 column (all i, weights/8) ----------------
            for it in range(2):
                za0 = spool.tile([128, 1], F32, tag=f"za0_{it}")
                nc.scalar.activation(za0[:, :], b0t[it][:, :], SIG)
                zp0 = spool.tile([128, 1], F32, tag=f"zp0_{it}")
                nc.scalar.activation(zp0[:, :], b1t[it][:, :], SIG)
                t0 = spool.tile([128, 1], F32, tag=f"t0_{it}")
                nc.vector.tensor_scalar(t0[:, :], za0[:, :],
                                        sv[:, 5:6], sv[:, 4:5],
                                        AL.mult, AL.add)
                nc.vector.scalar_tensor_tensor(t0[:, :], zp0[:, :],
                                               sv[:, 6:7], t0[:, :],
                                               AL.mult, AL.add)
                yl0 = spool.tile([128, 1], F32, tag=f"yl0_{it}")
                nc.vector.scalar_tensor_tensor(yl0[:, :], t0[:, :],
                                               LEAKY, t0[:, :],
                                               AL.mult, AL.max)
                yj0 = spool.tile([128, B], BF16, tag=f"yj0_{it}")
                nc.vector.tensor_copy(yj0[:, :],
                                      yl0[:, 0:1].broadcast_to([128, B]))
                for ht in range(NHT):
                    mm(ht, wc0[it][:, ht * 128:(ht + 1) * 128], yj0[:, :],
                       start=False, stop=False)

            # ---------------- main loop ----------------
            for c in range(NCHUNK):
                ar = arpool.tile([128, CH], BF16, tag="ar")
                nc.sync.dma_start(
                    ar[:, :],
                    d_arows[0:1, c * CH:(c + 1) * CH].broadcast_to([128, CH]))
                rr = arpool.tile([128, CH], BF16, tag="rr")
                nc.sync.dma_start(
                    rr[:, :],
                    d_rrows[0:1, c * CH:(c + 1) * CH].broadcast_to([128, CH]))

                for it in range(2):
                    w = wpool.tile([128, JC * H], BF16, tag="w")
                    r0 = (it * NCHUNK + c) * 128
                    nc.sync.dma_start(w[:, :], d_wmain[r0:r0 + 128, :])

                    za = zpool.tile([128, CH], BF16, tag="za")
                    nc.scalar.activation(za[:, :], ar[:, :], SIG,
                                         bias=b0t[it][:, :], scale=1.0)
                    zs = zpool.tile([128, CH], BF16, tag="zs")
                    nc.scalar.activation(zs[:, :], ar[:, :], SIG,
                                         bias=b0t[it][:, :], scale=-1.0)
                    zp = zpool.tile([128, CH], BF16, tag="zp")
                    nc.scalar.activation(zp[:, :], ar[:, :], SIG,
                                         bias=0.0, scale=b1t[it][:, :])
                    zd = zpool.tile([128, CH], BF16, tag="zd")
                    nc.scalar.activation(zd[:, :], rr[:, :], SIG,
                                         bias=0.0, scale=b1t[it][:, :])

                    ta = combpool.tile([128, CH], BF16, tag="ta")
                    nc.vector.tensor_scalar(ta[:, :], za[:, :],
                                            sv[:, 0:1], sv[:, 4:5],
                                            AL.mult, AL.add)
                    tb = combpool.tile([128, CH], BF16, tag="tb")
                    nc.vector.tensor_scalar(tb[:, :], zs[:, :],
                                            sv[:, 1:2], None, AL.mult)
                    tc2 = combpool.tile([128, CH], BF16, tag="tc2")
                    nc.vector.tensor_scalar(tc2[:, :], zp[:, :],
                                            sv[:, 2:3], None, AL.mult)
                    td = combpool.tile([128, CH], BF16, tag="td")
                    nc.vector.tensor_scalar(td[:, :], zd[:, :],
                                            sv[:, 3:4], None, AL.mult)
                    u1 = combpool.tile([128, CH], BF16, tag="u1")
                    nc.vector.tensor_add(u1[:, :], ta[:, :], tb[:, :])
                    u2 = combpool.tile([128, CH], BF16, tag="u2")
                    nc.gpsimd.tensor_add(u2[:, :], tc2[:, :], td[:, :])
                    y1 = combpool.tile([128, CH], BF16, tag="y1")
                    nc.vector.tensor_add(y1[:, :], u1[:, :], u2[:, :])
                    lk = combpool.tile([128, CH], BF16, tag="lk")
                    nc.vector.tensor_scalar(lk[:, :], y1[:, :],
                                            LEAKY, None, AL.mult)
                    yl = ylpool.tile([128, CH], BF16, tag="yl")
                    nc.vector.tensor_tensor(yl[:, :], y1[:, :], lk[:, :],
                                            AL.max)

                    last = (c == NCHUNK - 1) and (it == 1)
                    for jw in range(JC):
                        for ht in range(NHT):
                            mm(ht,
                               w[:, jw * H + ht * 128: jw * H + (ht + 1) * 128],
                               yl[:, jw * B:(jw + 1) * B],
                               start=False,
                               stop=last and (jw == JC - 1))

            # ---------------- write out partial y2 [H, B] ----------------
            for ht in range(NHT):
                ob = finpool.tile([128, B], F32, tag=f"ob{ht}", name=f"ob{ht}")
                nc.vector.tensor_copy(ob[:, :], ot[ht][:, :])
                nc.sync.dma_start(d_out[ht * 128:(ht + 1) * 128, :], ob[:, :])

    nc.finalize()
    return nc


_CACHED_NC = None


def _get_program():
    global _CACHED_NC
    if _CACHED_NC is None:
        _CACHED_NC = build_program()
    return _CACHED_NC


def make_in_maps(x1, x3, conv_w, conv_b, bn_gamma, bn_beta, bn_mean, bn_var,
                 fc_w, fc_b, out_w, out_b):
    x1 = np.asarray(x1, np.float32)
    x3 = np.asarray(x3, np.float32)
    fc_w = np.asarray(fc_w, np.float32)

    g = float(np.asarray(bn_gamma).reshape(-1)[0]) / float(
        np.sqrt(np.asarray(bn_var).reshape(-1)[0] + BN_EPS))
    s = np.asarray(conv_w, np.float32).reshape(-1) * g
    off = (float(np.asarray(conv_b).reshape(-1)[0])
           - float(np.asarray(bn_mean).reshape(-1)[0])) * g \
        + float(np.asarray(bn_beta).reshape(-1)[0])

    sv = np.zeros((128, 8), np.float32)
    sv[:, 0], sv[:, 1], sv[:, 2], sv[:, 3] = s[0], s[1], s[2], s[3]
    sv[:, 4] = off
    sv[:, 5] = s[0] + s[1]
    sv[:, 6] = s[2] + s[3]
    cv = np.full((128, 1), x3[-1], np.float32)

    b0 = np.concatenate([[0.0], x3]).astype(np.float32)  # (257,)
    b1 = np.concatenate([[1.0], x3]).astype(np.float32)
    b0m = b0[:256].reshape(256, 1).copy()
    b1m = b1[:256].reshape(256, 1).copy()

    # fc_w (H, 66049) with k = i*257+j  ->  W3 [i, j, h]
    w3 = np.ascontiguousarray(fc_w.reshape(H, NP, NP).transpose(1, 2, 0))

    x1T = np.ascontiguousarray(x1.T)                     # (256 j-1, 256 b)
    rT = (1.0 / (x1T + np.float32(EPS))).astype(np.float32)

    bf = np.dtype("bfloat16")
    in_maps = []
    for core in range(NCORE):
        jsl = slice(1 + JPC * core, 1 + JPC * (core + 1))  # j values
        xs = x1T[JPC * core: JPC * (core + 1), :]          # (32, 256)
        rsl = rT[JPC * core: JPC * (core + 1), :]

        # wmain [it, chunk, i(128), jw(8), h] -> [(2*4)*128, 8*512]
        wm = np.empty((2, NCHUNK, 128, JC, H), np.float32)
        for it in range(2):
            for ch in range(NCHUNK):
                j0 = 1 + JPC * core + ch * JC
                wm[it, ch] = w3[it * 128:(it + 1) * 128, j0:j0 + JC, :]
        wmain = np.ascontiguousarray(
            wm.reshape(2 * NCHUNK * 128, JC * H)).astype(bf)

        wcol0 = np.ascontiguousarray(w3[0:256, 0, :] / 8.0).astype(bf)

        wstrip = np.empty((JPC + 1, H), np.float32)
        wstrip[0] = w3[256, 0, :] / 8.0
        wstrip[1:] = w3[256, jsl, :]
        wstrip = wstrip.astype(bf)

        a0s = np.empty((JPC + 1, B), np.float32)
        a0s[0] = 0.0
        a0s[1:] = xs
        a1s = a0s.copy()
        a1s[0] = 1.0
        rstrip = np.empty((JPC + 1, B), np.float32)
        rstrip[0] = 1.0 / (1.0 + EPS)
        rstrip[1:] = rsl

        in_maps.append({
            "arows": np.ascontiguousarray(xs.reshape(1, JPC * B)).astype(bf),
            "rrows": np.ascontiguousarray(rsl.reshape(1, JPC * B)).astype(bf),
            "b0": b0m, "b1": b1m, "sv": sv, "cv": cv,
            "wmain": wmain, "wcol0": wcol0, "wstrip": wstrip,
            "a0s": a0s.astype(bf), "a1s": a1s.astype(bf),
            "rs": rstrip.astype(bf),
        })
    return in_maps


def kernel(**inputs):
    in_maps = make_in_maps(**inputs)
    nc = _get_program()
    res = run_bass_kernel_spmd(nc, in_maps, list(range(NCORE)))

    y2 = np.zeros((H, B), np.float32)
    for core in range(NCORE):
        y2 += np.asarray(res.results[core]["out"], np.float32)
    y2 = y2.T + np.asarray(inputs["fc_b"], np.float32).reshape(1, H)
    y2 = np.maximum(y2, 0.0)
    logits = y2 @ np.asarray(inputs["out_w"], np.float32).T \
        + np.asarray(inputs["out_b"], np.float32).reshape(1, C)
    return logits.astype(np.float32)


# revision 4
# speedup vs baseline: 2.0010x; 1.2084x over previous
"""Trainium2 Bass kernel for nn_MOAB_46273977647401.

Network (reference):
  x1 (256,256), x3 (256,) -> 4 outer sigmoid maps (256,257,257)
  -> 1x1 conv combine (4ch) + eval BN + leaky(0.1) -> (256, 66049)
  -> FC (66049 -> 512) + relu -> FC (512 -> 4)

Sharding: 8-way split of the FC contraction dim K = 257*257 by the j
(column) index: core c owns j in [1+32c, 33+32c); the j=0 column and the
i=256 strip's j=0 element are computed identically on every core against
1/8-scaled weights (outputs are partial sums, reduced on host).

Per core, z maps live in [i-partitions, (j,b)-free] layout. The a row
values (and their host-precomputed reciprocals) are DMA-broadcast from
DRAM to all 128 partitions as bf16; ScalarE computes the 4 sigmoid maps
with per-partition bias/scale; DVE+Pool do the conv+BN-folded combine +
leaky; PE accumulates out[h, b] PSUM tiles with the fc_w slab as the
stationary (lhsT) operand.

Host: sums the 8 partial (512, 256) outputs, applies fc bias + relu and
the tiny 512->4 output layer.
"""

import numpy as np

import concourse.bass as bass
import concourse.tile as tile
from concourse import bacc, mybir
from concourse.bass_utils import run_bass_kernel_spmd

F32 = mybir.dt.float32
BF16 = mybir.dt.bfloat16
AL = mybir.AluOpType
SIG = mybir.ActivationFunctionType.Sigmoid

B, N, H, C = 256, 256, 512, 4
NP = 257                  # N+1
NCORE = 8
JPC = 32                  # j columns per core (j in [1+32c, 33+32c))
JC = 8                    # j values per chunk
CH = JC * B               # 2048 free elems per chunk
NCHUNK = JPC // JC        # 4
NHT = H // 128            # 4 output h tiles
EPS = 1e-10
BN_EPS = 1e-5
LEAKY = 0.1


def build_program():
    nc = bacc.Bacc("TRN2", target_bir_lowering=False, debug=False, num_devices=8)

    # [a-rows | r-rows] per chunk, interleaved: chunk c occupies
    # cols [c*2*CH, (c+1)*2*CH) with a in the first CH and r in the second.
    d_axr = nc.dram_tensor("axr", [1, NCHUNK * 2 * CH], BF16,
                           kind="ExternalInput").ap()
    d_bcols = nc.dram_tensor("bcols", [2 * 128, 2], F32, kind="ExternalInput").ap()
    d_svcv = nc.dram_tensor("svcv", [128, 9], F32, kind="ExternalInput").ap()
    d_wmain = nc.dram_tensor("wmain", [2 * NCHUNK * 128, JC * H], BF16,
                             kind="ExternalInput").ap()
    d_wcol0 = nc.dram_tensor("wcol0", [128, 2 * H], BF16, kind="ExternalInput").ap()
    d_wstrip = nc.dram_tensor("wstrip", [JPC + 1, H], BF16, kind="ExternalInput").ap()
    d_stripa = nc.dram_tensor("stripa", [JPC + 1, 3 * B], BF16,
                              kind="ExternalInput").ap()
    d_out = nc.dram_tensor("out", [H, B], F32, kind="ExternalOutput").ap()

    JS = JPC + 1  # strip partitions

    with tile.TileContext(nc) as tc:
        with (
            tc.tile_pool(name="const", bufs=1) as cpool,
            tc.tile_pool(name="sp", bufs=1) as spool,
            tc.tile_pool(name="w", bufs=3) as wpool,
            tc.tile_pool(name="ar", bufs=2) as arpool,
            tc.tile_pool(name="z", bufs=2) as zpool,
            tc.tile_pool(name="comb", bufs=2) as combpool,
            tc.tile_pool(name="ylp", bufs=3) as ylpool,
            tc.tile_pool(name="fin", bufs=1) as finpool,
            tc.tile_pool(name="psO", bufs=1, space="PSUM") as psO,
        ):
            # ------- leading DMAs: first-chunk data before small consts ----
            axr0 = arpool.tile([128, 2 * CH], BF16, tag="axr")
            nc.sync.dma_start(axr0[:, :],
                              d_axr[0:1, 0:2 * CH].broadcast_to([128, 2 * CH]))

            bb = [cpool.tile([128, 2], F32, tag=f"bb_{k}", name=f"bb_{k}")
                  for k in range(2)]
            for k in range(2):
                nc.sync.dma_start(bb[k][:, :], d_bcols[k * 128:(k + 1) * 128, :])
            svcv = cpool.tile([128, 9], F32, tag="svcv")
            nc.sync.dma_start(svcv[:, :], d_svcv[:, :])
            sv = svcv  # cols 0..7; col 8 is cv
            b0t = [bb[k][:, 0:1] for k in range(2)]
            b1t = [bb[k][:, 1:2] for k in range(2)]
            cv = svcv[:, 8:9]

            w00 = wpool.tile([128, JC * H], BF16, tag="w")
            nc.sync.dma_start(w00[:, :], d_wmain[0:128, :])

            stripa = cpool.tile([JS, 3 * B], BF16, tag="stripa")
            nc.sync.dma_start(stripa[:, :], d_stripa[:, :])
            a0s = stripa[:, 0:B]
            a1s = stripa[:, B:2 * B]
            rs = stripa[:, 2 * B:3 * B]

            wc0 = cpool.tile([128, 2 * H], BF16, tag="wc0")
            nc.sync.dma_start(wc0[:, :], d_wcol0[:, :])
            wst = cpool.tile([JS, H], BF16, tag="wst")
            nc.sync.dma_start(wst[:, :], d_wstrip[:, :])

            # persistent PSUM accumulators out[h, b]
            ot = [psO.tile([128, B], F32, tag=f"ot{h}", name=f"ot{h}")
                  for h in range(NHT)]

            def mm(ht, lhsT, rhs, start, stop):
                nc.tensor.matmul(ot[ht][:, :], lhsT, rhs,
                                 start=start, stop=stop, skip_group_check=True)

            # ---------------- strip i=256 (j in core shard + j0/8) --------
            zas = spool.tile([JS, B], F32, tag="zas")
            nc.scalar.activation(zas[:, :], a0s, SIG,
                                 bias=cv[0:JS, :], scale=1.0)
            zss = spool.tile([JS, B], F32, tag="zss")
            nc.scalar.activation(zss[:, :], a0s, SIG,
                                 bias=cv[0:JS, :], scale=-1.0)
            zps = spool.tile([JS, B], F32, tag="zps")
            nc.scalar.activation(zps[:, :], a1s, SIG,
                                 bias=0.0, scale=cv[0:JS, :])
            zds = spool.tile([JS, B], F32, tag="zds")
            nc.scalar.activation(zds[:, :], rs, SIG,
                                 bias=0.0, scale=cv[0:JS, :])
            t1 = spool.tile([JS, B], F32, tag="t1")
            nc.vector.tensor_scalar(t1[:, :], zas[:, :],
                                    sv[0:JS, 0:1], sv[0:JS, 4:5],
                                    AL.mult, AL.add)
            nc.vector.scalar_tensor_tensor(t1[:, :], zss[:, :],
                                           sv[0:JS, 1:2], t1[:, :],
                                           AL.mult, AL.add)
            nc.vector.scalar_tensor_tensor(t1[:, :], zps[:, :],
                                           sv[0:JS, 2:3], t1[:, :],
                                           AL.mult, AL.add)
            nc.vector.scalar_tensor_tensor(t1[:, :], zds[:, :],
                                           sv[0:JS, 3:4], t1[:, :],
                                           AL.mult, AL.add)
            yls = spool.tile([JS, B], BF16, tag="yls")
            nc.vector.scalar_tensor_tensor(yls[:, :], t1[:, :],
                                           LEAKY, t1[:, :],
                                           AL.mult, AL.max)
            for ht in range(NHT):
                mm(ht, wst[:, ht * 128:(ht + 1) * 128], yls[:, :],
                   start=True, stop=False)

            # ---------------- j=0 column (all i, weights/8) ----------------
            for it in range(2):
                za0 = spool.tile([128, 1], F32, tag=f"za0_{it}")
                nc.scalar.activation(za0[:, :], b0t[it], SIG)
                zp0 = spool.tile([128, 1], F32, tag=f"zp0_{it}")
                nc.scalar.activation(zp0[:, :], b1t[it], SIG)
                t0 = spool.tile([128, 1], F32, tag=f"t0_{it}")
                nc.vector.tensor_scalar(t0[:, :], za0[:, :],
                                        sv[:, 5:6], sv[:, 4:5],
                                        AL.mult, AL.add)
                nc.vector.scalar_tensor_tensor(t0[:, :], zp0[:, :],
                                               sv[:, 6:7], t0[:, :],
                                               AL.mult, AL.add)
                yl0 = spool.tile([128, 1], F32, tag=f"yl0_{it}")
                nc.vector.scalar_tensor_tensor(yl0[:, :], t0[:, :],
                                               LEAKY, t0[:, :],
                                               AL.mult, AL.max)
                yj0 = spool.tile([128, B], BF16, tag=f"yj0_{it}")
                nc.vector.tensor_copy(yj0[:, :],
                                      yl0[:, 0:1].broadcast_to([128, B]))
                for ht in range(NHT):
                    mm(ht, wc0[:, it * H + ht * 128: it * H + (ht + 1) * 128],
                       yj0[:, :], start=False, stop=False)

            # ---------------- main loop ----------------
            for c in range(NCHUNK):
                if c == 0:
                    axr = axr0
                else:
                    axr = arpool.tile([128, 2 * CH], BF16, tag="axr")
                    nc.sync.dma_start(
                        axr[:, :],
                        d_axr[0:1, c * 2 * CH:(c + 1) * 2 * CH]
                        .broadcast_to([128, 2 * CH]))
                ar = axr[:, 0:CH]
                rr = axr[:, CH:2 * CH]

                for it in range(2):
                    if c == 0 and it == 0:
                        w = w00
                    else:
                        w = wpool.tile([128, JC * H], BF16, tag="w")
                        r0 = (it * NCHUNK + c) * 128
                        nc.sync.dma_start(w[:, :], d_wmain[r0:r0 + 128, :])

                    za = zpool.tile([128, CH], BF16, tag="za")
                    nc.scalar.activation(za[:, :], ar, SIG,
                                         bias=b0t[it], scale=1.0)
                    zs = zpool.tile([128, CH], BF16, tag="zs")
                    nc.scalar.activation(zs[:, :], ar, SIG,
                                         bias=b0t[it], scale=-1.0)
                    zp = zpool.tile([128, CH], BF16, tag="zp")
                    nc.scalar.activation(zp[:, :], ar, SIG,
                                         bias=0.0, scale=b1t[it])
                    zd = zpool.tile([128, CH], BF16, tag="zd")
                    nc.scalar.activation(zd[:, :], rr, SIG,
                                         bias=0.0, scale=b1t[it])

                    # combine: ta,td,u1,u2,y1,lk,yl on DVE; tb,tc2 on Pool
                    ta = combpool.tile([128, CH], BF16, tag="ta")
                    nc.vector.tensor_scalar(ta[:, :], za[:, :],
                                            sv[:, 0:1], sv[:, 4:5],
                                            AL.mult, AL.add)
                    tb = combpool.tile([128, CH], BF16, tag="tb")
                    nc.gpsimd.tensor_scalar(tb[:, :], zs[:, :],
                                            sv[:, 1:2], None, AL.mult)
                    tc2 = combpool.tile([128, CH], BF16, tag="tc2")
                    nc.gpsimd.tensor_scalar(tc2[:, :], zp[:, :],
                                            sv[:, 2:3], None, AL.mult)
                    td = combpool.tile([128, CH], BF16, tag="td")
                    nc.vector.tensor_scalar(td[:, :], zd[:, :],
                                            sv[:, 3:4], None, AL.mult)
                    u1 = combpool.tile([128, CH], BF16, tag="u1")
                    nc.vector.tensor_add(u1[:, :], ta[:, :], tb[:, :])
                    u2 = combpool.tile([128, CH], BF16, tag="u2")
                    nc.vector.tensor_add(u2[:, :], tc2[:, :], td[:, :])
                    y1 = combpool.tile([128, CH], BF16, tag="y1")
                    nc.vector.tensor_add(y1[:, :], u1[:, :], u2[:, :])
                    lk = combpool.tile([128, CH], BF16, tag="lk")
                    nc.vector.tensor_scalar(lk[:, :], y1[:, :],
                                            LEAKY, None, AL.mult)
                    yl = ylpool.tile([128, CH], BF16, tag="yl")
                    nc.vector.tensor_tensor(yl[:, :], y1[:, :], lk[:, :],
                                            AL.max)

                    last = (c == NCHUNK - 1) and (it == 1)
                    for jw in range(JC):
                        for ht in range(NHT):
                            mm(ht,
                               w[:, jw * H + ht * 128: jw * H + (ht + 1) * 128],
                               yl[:, jw * B:(jw + 1) * B],
                               start=False,
                               stop=last and (jw == JC - 1))

            # ---------------- write out partial y2 [H, B] ----------------
            for ht in range(NHT):
                ob = finpool.tile([128, B], F32, tag=f"ob{ht}", name=f"ob{ht}")
                nc.scalar.copy(ob[:, :], ot[ht][:, :])
                nc.sync.dma_start(d_out[ht * 128:(ht + 1) * 128, :], ob[:, :])

    nc.finalize()
    return nc


_CACHED_NC = None


def _get_program():
    global _CACHED_NC
    if _CACHED_NC is None:
        _CACHED_NC = build_program()
    return _CACHED_NC


def make_in_maps(x1, x3, conv_w, conv_b, bn_gamma, bn_beta, bn_mean, bn_var,
                 fc_w, fc_b, out_w, out_b):
    x1 = np.asarray(x1, np.float32)
    x3 = np.asarray(x3, np.float32)
    fc_w = np.asarray(fc_w, np.float32)

    g = float(np.asarray(bn_gamma).reshape(-1)[0]) / float(
        np.sqrt(np.asarray(bn_var).reshape(-1)[0] + BN_EPS))
    s = np.asarray(conv_w, np.float32).reshape(-1) * g
    off = (float(np.asarray(conv_b).reshape(-1)[0])
           - float(np.asarray(bn_mean).reshape(-1)[0])) * g \
        + float(np.asarray(bn_beta).reshape(-1)[0])

    svcv = np.zeros((128, 9), np.float32)
    svcv[:, 0], svcv[:, 1], svcv[:, 2], svcv[:, 3] = s[0], s[1], s[2], s[3]
    svcv[:, 4] = off
    svcv[:, 5] = s[0] + s[1]
    svcv[:, 6] = s[2] + s[3]
    svcv[:, 8] = x3[-1]

    b0 = np.concatenate([[0.0], x3]).astype(np.float32)  # (257,)
    b1 = np.concatenate([[1.0], x3]).astype(np.float32)
    bcols = np.stack([b0[:256], b1[:256]], axis=1).astype(np.float32)  # (256,2)

    # fc_w (H, 66049) with k = i*257+j  ->  W3 [i, j, h]
    w3 = np.ascontiguousarray(fc_w.reshape(H, NP, NP).transpose(1, 2, 0))

    x1T = np.ascontiguousarray(x1.T)                     # (256 j-1, 256 b)
    rT = (1.0 / (x1T + np.float32(EPS))).astype(np.float32)

    bf = np.dtype("bfloat16")
    in_maps = []
    for core in range(NCORE):
        jsl = slice(1 + JPC * core, 1 + JPC * (core + 1))  # j values
        xs = x1T[JPC * core: JPC * (core + 1), :]          # (32, 256)
        rsl = rT[JPC * core: JPC * (core + 1), :]

        # axr: per chunk c: [a rows (JC*B) | r rows (JC*B)]
        axr = np.empty((NCHUNK, 2, JC * B), np.float32)
        for ch in range(NCHUNK):
            axr[ch, 0] = xs[ch * JC:(ch + 1) * JC].reshape(-1)
            axr[ch, 1] = rsl[ch * JC:(ch + 1) * JC].reshape(-1)
        axr = axr.reshape(1, NCHUNK * 2 * JC * B).astype(bf)

        # wmain [it, chunk, i(128), jw(8), h] -> [(2*4)*128, 8*512]
        wm = np.empty((2, NCHUNK, 128, JC, H), np.float32)
        for it in range(2):
            for ch in range(NCHUNK):
                j0 = 1 + JPC * core + ch * JC
                wm[it, ch] = w3[it * 128:(it + 1) * 128, j0:j0 + JC, :]
        wmain = np.ascontiguousarray(
            wm.reshape(2 * NCHUNK * 128, JC * H)).astype(bf)

        # wcol0 [128 i, 2 it * H]
        wcol0 = np.concatenate(
            [w3[0:128, 0, :] / 8.0, w3[128:256, 0, :] / 8.0],
            axis=1).astype(bf)

        wstrip = np.empty((JPC + 1, H), np.float32)
        wstrip[0] = w3[256, 0, :] / 8.0
        wstrip[1:] = w3[256, jsl, :]
        wstrip = wstrip.astype(bf)

        stripa = np.empty((JPC + 1, 3 * B), np.float32)
        stripa[0, 0:B] = 0.0                # a0 at j=0
        stripa[1:, 0:B] = xs
        stripa[0, B:2 * B] = 1.0            # a1 at j=0
        stripa[1:, B:2 * B] = xs
        stripa[0, 2 * B:] = 1.0 / (1.0 + EPS)
        stripa[1:, 2 * B:] = rsl

        in_maps.append({
            "axr": axr, "bcols": bcols, "svcv": svcv,
            "wmain": wmain, "wcol0": wcol0, "wstrip": wstrip,
            "stripa": stripa.astype(bf),
        })
    return in_maps


def kernel(**inputs):
    in_maps = make_in_maps(**inputs)
    nc = _get_program()
    res = run_bass_kernel_spmd(nc, in_maps, list(range(NCORE)))

    y2 = np.zeros((H, B), np.float32)
    for core in range(NCORE):
        y2 += np.asarray(res.results[core]["out"], np.float32)
    y2 = y2.T + np.asarray(inputs["fc_b"], np.float32).reshape(1, H)
    y2 = np.maximum(y2, 0.0)
    logits = y2 @ np.asarray(inputs["out_w"], np.float32).T \
        + np.asarray(inputs["out_b"], np.float32).reshape(1, C)
    return logits.astype(np.float32)


# revision 11
# speedup vs baseline: 2.0289x; 1.0140x over previous
"""Trainium2 Bass kernel for nn_MOAB_46273977647401.

Network (reference):
  x1 (256,256), x3 (256,) -> 4 outer sigmoid maps (256,257,257)
  -> 1x1 conv combine (4ch) + eval BN + leaky(0.1) -> (256, 66049)
  -> FC (66049 -> 512) + relu -> FC (512 -> 4)

Sharding: 8-way split of the FC contraction dim K = 257*257 by the j
(column) index: core c owns j in [1+32c, 33+32c); the j=0 column and the
i=256 strip's j=0 element are computed identically on every core against
1/8-scaled weights (outputs are partial sums, reduced on host).

Per core, z maps live in [i-partitions, (j,b)-free] layout. The a row
values (and their host-precomputed reciprocals) are DMA-broadcast from
DRAM to all 128 partitions as bf16; ScalarE computes the 4 sigmoid maps
with per-partition bias/scale; DVE+Pool do the conv+BN-folded combine +
leaky; PE accumulates out[h, b] PSUM tiles with the fc_w slab as the
stationary (lhsT) operand.

Host: sums the 8 partial (512, 256) outputs, applies fc bias + relu and
the tiny 512->4 output layer.
"""

import numpy as np

import concourse.bass as bass
import concourse.tile as tile
from concourse import bacc, mybir
from concourse.bass_utils import run_bass_kernel_spmd

F32 = mybir.dt.float32
BF16 = mybir.dt.bfloat16
AL = mybir.AluOpType
SIG = mybir.ActivationFunctionType.Sigmoid

B, N, H, C = 256, 256, 512, 4
NP = 257                  # N+1
NCORE = 8
JPC = 32                  # j columns per core (j in [1+32c, 33+32c))
JC = 8                    # j values per chunk
CH = JC * B               # 2048 free elems per chunk
NCHUNK = JPC // JC        # 4
NHT = H // 128            # 4 output h tiles
EPS = 1e-10
BN_EPS = 1e-5
LEAKY = 0.1


def build_program():
    nc = bacc.Bacc("TRN2", target_bir_lowering=False, debug=False, num_devices=8)

    # a-rows and r-rows, one CH-sized block per chunk.
    d_arows = nc.dram_tensor("arows", [1, NCHUNK * CH], BF16,
                             kind="ExternalInput").ap()
    d_rrows = nc.dram_tensor("rrows", [1, NCHUNK * CH], BF16,
                             kind="ExternalInput").ap()
    d_bcols = nc.dram_tensor("bcols", [2 * 128, 2], F32, kind="ExternalInput").ap()
    d_svcv = nc.dram_tensor("svcv", [128, 9], F32, kind="ExternalInput").ap()
    d_wmain = nc.dram_tensor("wmain", [2 * NCHUNK * 128, JC * H], BF16,
                             kind="ExternalInput").ap()
    d_wcol0 = nc.dram_tensor("wcol0", [128, 2 * H], BF16, kind="ExternalInput").ap()
    d_wstrip = nc.dram_tensor("wstrip", [JPC + 1, H], BF16, kind="ExternalInput").ap()
    d_stripa = nc.dram_tensor("stripa", [JPC + 1, 3 * B], BF16,
                              kind="ExternalInput").ap()
    d_out = nc.dram_tensor("out", [128, NHT * B], F32, kind="ExternalOutput").ap()

    JS = JPC + 1  # strip partitions

    with tile.TileContext(nc) as tc:
        with (
            tc.tile_pool(name="const", bufs=1) as cpool,
            tc.tile_pool(name="sp", bufs=1) as spool,
            tc.tile_pool(name="w", bufs=3) as wpool,
            tc.tile_pool(name="ar", bufs=2) as arpool,
            tc.tile_pool(name="z", bufs=2) as zpool,
            tc.tile_pool(name="comb", bufs=2) as combpool,
            tc.tile_pool(name="ylp", bufs=3) as ylpool,
            tc.tile_pool(name="fin", bufs=1) as finpool,
            tc.tile_pool(name="psO", bufs=1, space="PSUM") as psO,
        ):
            # ------- leading DMAs: small consts, then first-chunk data -----
            bb = [cpool.tile([128, 2], F32, tag=f"bb_{k}", name=f"bb_{k}")
                  for k in range(2)]
            for k in range(2):
                nc.sync.dma_start(bb[k][:, :], d_bcols[k * 128:(k + 1) * 128, :])
            svcv = cpool.tile([128, 9], F32, tag="svcv")
            nc.sync.dma_start(svcv[:, :], d_svcv[:, :])
            sv = svcv  # cols 0..7; col 8 is cv
            b0t = [bb[k][:, 0:1] for k in range(2)]
            b1t = [bb[k][:, 1:2] for k in range(2)]
            cv = svcv[:, 8:9]

            ar0 = arpool.tile([128, CH], BF16, tag="ar")
            nc.sync.dma_start(ar0[:, :],
                              d_arows[0:1, 0:CH].broadcast_to([128, CH]))
            rr0 = arpool.tile([128, CH], BF16, tag="rr")
            nc.sync.dma_start(rr0[:, :],
                              d_rrows[0:1, 0:CH].broadcast_to([128, CH]))

            w00 = wpool.tile([128, JC * H], BF16, tag="w")
            nc.sync.dma_start(w00[:, :], d_wmain[0:128, :])

            stripa = cpool.tile([JS, 3 * B], BF16, tag="stripa")
            nc.sync.dma_start(stripa[:, :], d_stripa[:, :])
            a0s = stripa[:, 0:B]
            a1s = stripa[:, B:2 * B]
            rs = stripa[:, 2 * B:3 * B]

            wc0 = cpool.tile([128, 2 * H], BF16, tag="wc0")
            nc.sync.dma_start(wc0[:, :], d_wcol0[:, :])
            wst = cpool.tile([JS, H], BF16, tag="wst")
            nc.sync.dma_start(wst[:, :], d_wstrip[:, :])

            # persistent PSUM accumulators out[h, b]
            ot = [psO.tile([128, B], F32, tag=f"ot{h}", name=f"ot{h}")
                  for h in range(NHT)]

            def mm(ht, lhsT, rhs, start, stop):
                nc.tensor.matmul(ot[ht][:, :], lhsT, rhs,
                                 start=start, stop=stop, skip_group_check=True)

            # ---------------- strip i=256 (j in core shard + j0/8) --------
            zas = spool.tile([JS, B], F32, tag="zas")
            nc.scalar.activation(zas[:, :], a0s, SIG,
                                 bias=cv[0:JS, :], scale=1.0)
            zss = spool.tile([JS, B], F32, tag="zss")
            nc.scalar.activation(zss[:, :], a0s, SIG,
                                 bias=cv[0:JS, :], scale=-1.0)
            zps = spool.tile([JS, B], F32, tag="zps")
            nc.scalar.activation(zps[:, :], a1s, SIG,
                                 bias=0.0, scale=cv[0:JS, :])
            zds = spool.tile([JS, B], F32, tag="zds")
            nc.scalar.activation(zds[:, :], rs, SIG,
                                 bias=0.0, scale=cv[0:JS, :])
            t1 = spool.tile([JS, B], F32, tag="t1")
            nc.vector.tensor_scalar(t1[:, :], zas[:, :],
                                    sv[0:JS, 0:1], sv[0:JS, 4:5],
                                    AL.mult, AL.add)
            nc.vector.scalar_tensor_tensor(t1[:, :], zss[:, :],
                                           sv[0:JS, 1:2], t1[:, :],
                                           AL.mult, AL.add)
            nc.vector.scalar_tensor_tensor(t1[:, :], zps[:, :],
                                           sv[0:JS, 2:3], t1[:, :],
                                           AL.mult, AL.add)
            nc.vector.scalar_tensor_tensor(t1[:, :], zds[:, :],
                                           sv[0:JS, 3:4], t1[:, :],
                                           AL.mult, AL.add)
            yls = spool.tile([JS, B], BF16, tag="yls")
            nc.vector.scalar_tensor_tensor(yls[:, :], t1[:, :],
                                           LEAKY, t1[:, :],
                                           AL.mult, AL.max)
            for ht in range(NHT):
                mm(ht, wst[:, ht * 128:(ht + 1) * 128], yls[:, :],
                   start=True, stop=False)

            # ---------------- j=0 column (all i, weights/8) ----------------
            for it in range(2):
                za0 = spool.tile([128, 1], F32, tag=f"za0_{it}")
                nc.scalar.activation(za0[:, :], b0t[it], SIG)
                zp0 = spool.tile([128, 1], F32, tag=f"zp0_{it}")
                nc.scalar.activation(zp0[:, :], b1t[it], SIG)
                t0 = spool.tile([128, 1], F32, tag=f"t0_{it}")
                nc.vector.tensor_scalar(t0[:, :], za0[:, :],
                                        sv[:, 5:6], sv[:, 4:5],
                                        AL.mult, AL.add)
                nc.vector.scalar_tensor_tensor(t0[:, :], zp0[:, :],
                                               sv[:, 6:7], t0[:, :],
                                               AL.mult, AL.add)
                yl0 = spool.tile([128, 1], F32, tag=f"yl0_{it}")
                nc.vector.scalar_tensor_tensor(yl0[:, :], t0[:, :],
                                               LEAKY, t0[:, :],
                                               AL.mult, AL.max)
                yj0 = spool.tile([128, B], BF16, tag=f"yj0_{it}")
                nc.vector.tensor_copy(yj0[:, :],
                                      yl0[:, 0:1].broadcast_to([128, B]))
                for ht in range(NHT):
                    mm(ht, wc0[:, it * H + ht * 128: it * H + (ht + 1) * 128],
                       yj0[:, :], start=False, stop=False)

            # ---------------- main loop ----------------
            HCH = CH // 2   # combine/matmul half-granularity
            JH = JC // 2
            for c in range(NCHUNK):
                if c == 0:
                    ar, rr = ar0, rr0
                else:
                    ar = arpool.tile([128, CH], BF16, tag="ar")
                    nc.sync.dma_start(
                        ar[:, :],
                        d_arows[0:1, c * CH:(c + 1) * CH]
                        .broadcast_to([128, CH]))
                    rr = arpool.tile([128, CH], BF16, tag="rr")
                    nc.sync.dma_start(
                        rr[:, :],
                        d_rrows[0:1, c * CH:(c + 1) * CH]
                        .broadcast_to([128, CH]))

                for it in range(2):
                    if c == 0 and it == 0:
                        w = w00
                    else:
                        w = wpool.tile([128, JC * H], BF16, tag="w")
                        r0 = (it * NCHUNK + c) * 128
                        nc.sync.dma_start(w[:, :], d_wmain[r0:r0 + 128, :])

                    za = zpool.tile([128, CH], BF16, tag="za")
                    nc.scalar.activation(za[:, :], ar[:, :], SIG,
                                         bias=b0t[it], scale=1.0)
                    zs = zpool.tile([128, CH], BF16, tag="zs")
                    nc.scalar.activation(zs[:, :], ar[:, :], SIG,
                                         bias=b0t[it], scale=-1.0)
                    zp = zpool.tile([128, CH], BF16, tag="zp")
                    nc.scalar.activation(zp[:, :], ar[:, :], SIG,
                                         bias=0.0, scale=b1t[it])
                    zd = zpool.tile([128, CH], BF16, tag="zd")
                    nc.scalar.activation(zd[:, :], rr[:, :], SIG,
                                         bias=0.0, scale=b1t[it])

                    last = (c == NCHUNK - 1) and (it == 1)
                    for hf in range(2):
                        sl = slice(hf * HCH, (hf + 1) * HCH)
                        # combine: ta,td,u1,u2,y1,lk,yl on DVE; tb,tc2 on Pool
                        ta = combpool.tile([128, HCH], BF16, tag="ta")
                        nc.vector.tensor_scalar(ta[:, :], za[:, sl],
                                                sv[:, 0:1], sv[:, 4:5],
                                                AL.mult, AL.add)
                        tb = combpool.tile([128, HCH], BF16, tag="tb")
                        nc.gpsimd.tensor_scalar(tb[:, :], zs[:, sl],
                                                sv[:, 1:2], None, AL.mult)
                        tc2 = combpool.tile([128, HCH], BF16, tag="tc2")
                        nc.gpsimd.tensor_scalar(tc2[:, :], zp[:, sl],
                                                sv[:, 2:3], None, AL.mult)
                        td = combpool.tile([128, HCH], BF16, tag="td")
                        nc.vector.tensor_scalar(td[:, :], zd[:, sl],
                                                sv[:, 3:4], None, AL.mult)
                        u1 = combpool.tile([128, HCH], BF16, tag="u1")
                        nc.vector.tensor_add(u1[:, :], ta[:, :], tb[:, :])
                        u2 = combpool.tile([128, HCH], BF16, tag="u2")
                        nc.vector.tensor_add(u2[:, :], tc2[:, :], td[:, :])
                        y1 = combpool.tile([128, HCH], BF16, tag="y1")
                        nc.vector.tensor_add(y1[:, :], u1[:, :], u2[:, :])
                        lk = combpool.tile([128, HCH], BF16, tag="lk")
                        nc.vector.tensor_scalar(lk[:, :], y1[:, :],
                                                LEAKY, None, AL.mult)
                        yl = ylpool.tile([128, HCH], BF16, tag="yl")
                        nc.vector.tensor_tensor(yl[:, :], y1[:, :], lk[:, :],
                                                AL.max)

                        for jw in range(JH):
                            j = hf * JH + jw
                            for ht in range(NHT):
                                mm(ht,
                                   w[:, j * H + ht * 128: j * H + (ht + 1) * 128],
                                   yl[:, jw * B:(jw + 1) * B],
                                   start=False,
                                   stop=(last and hf == 1 and jw == JH - 1))

            # ---------------- write out partial y2 [128, 4*B] ----------------
            ob = finpool.tile([128, NHT * B], F32, tag="ob")
            for ht in range(NHT):
                osl = ob[:, ht * B:(ht + 1) * B]
                if ht % 2 == 0:
                    nc.scalar.copy(osl, ot[ht][:, :])
                else:
                    nc.vector.tensor_copy(osl, ot[ht][:, :])
            nc.sync.dma_start(d_out[:, :], ob[:, :])

    nc.finalize()
    return nc


_CACHED_NC = None


def _get_program():
    global _CACHED_NC
    if _CACHED_NC is None:
        _CACHED_NC = build_program()
    return _CACHED_NC


def make_in_maps(x1, x3, conv_w, conv_b, bn_gamma, bn_beta, bn_mean, bn_var,
                 fc_w, fc_b, out_w, out_b):
    x1 = np.asarray(x1, np.float32)
    x3 = np.asarray(x3, np.float32)
    fc_w = np.asarray(fc_w, np.float32)

    g = float(np.asarray(bn_gamma).reshape(-1)[0]) / float(
        np.sqrt(np.asarray(bn_var).reshape(-1)[0] + BN_EPS))
    s = np.asarray(conv_w, np.float32).reshape(-1) * g
    off = (float(np.asarray(conv_b).reshape(-1)[0])
           - float(np.asarray(bn_mean).reshape(-1)[0])) * g \
        + float(np.asarray(bn_beta).reshape(-1)[0])

    svcv = np.zeros((128, 9), np.float32)
    svcv[:, 0], svcv[:, 1], svcv[:, 2], svcv[:, 3] = s[0], s[1], s[2], s[3]
    svcv[:, 4] = off
    svcv[:, 5] = s[0] + s[1]
    svcv[:, 6] = s[2] + s[3]
    svcv[:, 8] = x3[-1]

    b0 = np.concatenate([[0.0], x3]).astype(np.float32)  # (257,)
    b1 = np.concatenate([[1.0], x3]).astype(np.float32)
    bcols = np.stack([b0[:256], b1[:256]], axis=1).astype(np.float32)  # (256,2)

    # fc_w (H, 66049) with k = i*257+j  ->  W3 [i, j, h]
    w3 = np.ascontiguousarray(fc_w.reshape(H, NP, NP).transpose(1, 2, 0))

    x1T = np.ascontiguousarray(x1.T)                     # (256 j-1, 256 b)
    rT = (1.0 / (x1T + np.float32(EPS))).astype(np.float32)

    bf = np.dtype("bfloat16")
    in_maps = []
    for core in range(NCORE):
        jsl = slice(1 + JPC * core, 1 + JPC * (core + 1))  # j values
        xs = x1T[JPC * core: JPC * (core + 1), :]          # (32, 256)
        rsl = rT[JPC * core: JPC * (core + 1), :]

        arows = np.ascontiguousarray(xs.reshape(1, JPC * B)).astype(bf)
        rrows = np.ascontiguousarray(rsl.reshape(1, JPC * B)).astype(bf)

        # wmain [it, chunk, i(128), jw(8), h] -> [(2*4)*128, 8*512]
        wm = np.empty((2, NCHUNK, 128, JC, H), np.float32)
        for it in range(2):
            for ch in range(NCHUNK):
                j0 = 1 + JPC * core + ch * JC
                wm[it, ch] = w3[it * 128:(it + 1) * 128, j0:j0 + JC, :]
        wmain = np.ascontiguousarray(
            wm.reshape(2 * NCHUNK * 128, JC * H)).astype(bf)

        # wcol0 [128 i, 2 it * H]
        wcol0 = np.concatenate(
            [w3[0:128, 0, :] / 8.0, w3[128:256, 0, :] / 8.0],
            axis=1).astype(bf)

        wstrip = np.empty((JPC + 1, H), np.float32)
        wstrip[0] = w3[256, 0, :] / 8.0
        wstrip[1:] = w3[256, jsl, :]
        wstrip = wstrip.astype(bf)

        stripa = np.empty((JPC + 1, 3 * B), np.float32)
        stripa[0, 0:B] = 0.0                # a0 at j=0
        stripa[1:, 0:B] = xs
        stripa[0, B:2 * B] = 1.0            # a1 at j=0
        stripa[1:, B:2 * B] = xs
        stripa[0, 2 * B:] = 1.0 / (1.0 + EPS)
        stripa[1:, 2 * B:] = rsl

        in_maps.append({
            "arows": arows, "rrows": rrows, "bcols": bcols, "svcv": svcv,
            "wmain": wmain, "wcol0": wcol0, "wstrip": wstrip,
            "stripa": stripa.astype(bf),
        })
    return in_maps


def kernel(**inputs):
    in_maps = make_in_maps(**inputs)
    nc = _get_program()
    res = run_bass_kernel_spmd(nc, in_maps, list(range(NCORE)))

    acc = np.zeros((128, NHT * B), np.float32)
    for core in range(NCORE):
        acc += np.asarray(res.results[core]["out"], np.float32)
    # acc[p, ht*B + b] = y2[ht*128 + p, b]
    y2 = acc.reshape(128, NHT, B).transpose(1, 0, 2).reshape(H, B)
    y2 = y2.T + np.asarray(inputs["fc_b"], np.float32).reshape(1, H)
    y2 = np.maximum(y2, 0.0)
    logits = y2 @ np.asarray(inputs["out_w"], np.float32).T \
        + np.asarray(inputs["out_b"], np.float32).reshape(1, C)
    return logits.astype(np.float32)
